# revision 50
# baseline (speedup 1.0000x reference)
"""Trainium2 Bass kernel for nn_Attn_48137993453608.

Module: Y = X@W1.T+b1 -> split Q,K,V -> w = softmax((Q_h^T K_h)/sqrt(S))
        (attention over the DH=64 dim, contracting S) -> out = w @ V_h^T
        -> raw memory-order reshape [B,H,DH,S]->[B,S,D] -> @ W2.T + b2.

Sharding: 8 cores = 4 batch x 2 head-groups (8 heads each). Each core owns a
contiguous [1024, 1024] block of the output (rows i = 128*h + 2*d + (s>=1024)
for its heads), so no collectives are needed.

Key reassociation: the final projection contracts the attention output over
j = s mod 1024, and the attention output is linear in V, so

  F_un[c2, n] = sum_j OT_un[j, c2] W2T[j, n]
              = sum_e expw[e, c2] * G[e, n],
  G_{p,half}[e, n] = sum_j V[half*1024+j, p*128+e] * W2T[j, n].

G is softmax-independent, so nearly all of the output-projection FLOPs run
inside phase 1's dense matmul stream; only a single [128x128]x[128x512]
matmul per (pair, half, nh) remains after the softmax.

Per-core dataflow:
  1. Y[s, :1536] = Xb @ Wqkv.T + b     (Q | K | V columns, local heads)
     Q,K columns feed wT; V columns are stored per s-tile (f32r).
  2. wT_h[e, d] = sum_s K_h[s,e] Q_h[s,d]   (PSUM accum over all s)
  3. G pass (still phase 1 PE work): G = V^T-blocks @ W2T-blocks
  4. expwT = exp(wT / sqrt(S))        (no max-sub: |logits| <= ~6)
     block-diag per head pair; Z via ones-matmul; rZ = 1/Z
  5. F_un = expw2^T-matmul against G; F = F_un * rZ[c2] + b2
  6. scatter F rows to the output block: r = 256*p + 128*g + 2*d + half

Precision: X/Wqkv in bf16 (PE streams 1 row/cycle regardless of dtype, so
bf16 only halves DMA/SBUF); logits are soft (|logit| <= ~6) so the ~0.3%
softmax-weight perturbation is benign.  wT runs in fp32 from the f32 Y,
V/G/W2 path in float32r with one bf16 rounding at G.
"""

import os
import sys

for _p in ("/opt/trn_rl_repo",):
    if _p not in sys.path and os.path.isdir(_p):
        sys.path.insert(0, _p)

import ml_dtypes
import numpy as np

import concourse.bass as bass
import concourse.bacc as bacc
import concourse.mybir as mybir
import concourse.tile as tile
from concourse.bass_utils import run_bass_kernel_spmd

B, S, D, H = 4, 2048, 1024, 16
DH = D // H          # 64
NH = 8               # heads per core
SCALE = 1.0 / float(np.sqrt(np.float32(S)))

F32 = mybir.dt.float32
F32R = mybir.dt.float32r
BF16 = mybir.dt.bfloat16

S_CHUNK = 256                 # s columns of X^T staged per iteration
N_SCHUNKS = S // S_CHUNK      # 8
ST_PER_CHUNK = S_CHUNK // 128 # 2


def build_nc():
    nc = bacc.Bacc("TRN2", target_bir_lowering=False, debug=False)

    xbt = nc.dram_tensor("xbt", [D, S], BF16, kind="ExternalInput")        # X[b].T
    wqkvt = nc.dram_tensor("wqkvt", [D, 1536], BF16, kind="ExternalInput") # [D, Q|K|V rows]
    bqkv = nc.dram_tensor("bqkv", [1, 1536], F32, kind="ExternalInput")
    w2t = nc.dram_tensor("w2t", [D, 1024], F32R, kind="ExternalInput")     # W2.T
    b2 = nc.dram_tensor("b2", [1, 1024], F32, kind="ExternalInput")
    out = nc.dram_tensor("out", [1024, 1024], F32, kind="ExternalOutput")

    xbt_v = xbt[:].rearrange("(kb p) s -> p kb s", p=128)      # [128, 8, 2048]
    # output rows r = 256*p + 128*g + 2*d + half
    out_v = out[:].rearrange("(p g d h) n -> p g d h n", p=4, g=2, d=64, h=2)

    with tile.TileContext(nc) as tc:
        with (
            tc.tile_pool(name="const", bufs=1) as const,
            tc.tile_pool(name="xin", bufs=3) as xin,
            tc.tile_pool(name="ywork", bufs=8) as ywork,
            tc.tile_pool(name="vstore", bufs=1) as vstore,
            tc.tile_pool(name="attn", bufs=1) as attn,
            tc.tile_pool(name="fout", bufs=4) as fout,
            tc.tile_pool(name="psacc", bufs=3, space="PSUM") as psacc,
            tc.tile_pool(name="pswt", bufs=1, space="PSUM") as pswt,
            tc.tile_pool(name="psg", bufs=2, space="PSUM") as psg,
            tc.tile_pool(name="psf", bufs=2, space="PSUM") as psf,
        ):
            # ---------------- phase-1 loads (w2/b2 deferred) --------------
            xbt_tiles = []
            xbt_sb0 = xin.tile([128, 8, S_CHUNK], BF16, tag="xbt")
            nc.sync.dma_start(out=xbt_sb0[:], in_=xbt_v[:, :, 0:S_CHUNK])
            xbt_tiles.append(xbt_sb0)

            # Per-kb loads keep >=3KB contiguous bursts; first QK matmul can
            # start after just the kb=0 block.
            wqkv_sb = const.tile([128, 8, 1536], BF16)
            for kb in range(8):
                nc.scalar.dma_start(out=wqkv_sb[:, kb, :],
                                    in_=wqkvt[kb * 128:(kb + 1) * 128, :])

            b_bc = const.tile([128, 1536], F32)
            nc.gpsimd.dma_start(out=b_bc[:], in_=bqkv[:].to_broadcast((128, 1536)))

            ones_sb = const.tile([128, 1], BF16)
            nc.vector.memset(ones_sb[:], 1.0)

            # V (natural [s, vrow] layout) persists until the G pass;
            # wT accumulates across all s.
            v_sb = vstore.tile([128, 16, 512], F32R)
            psum_wt = pswt.tile([128, 512], F32)        # [e2(g*64+e), pair*128+c2]

            # ---------------- phase 1: QKV projection + wT ----------------
            for sc in range(N_SCHUNKS):
                if sc < len(xbt_tiles):
                    xbt_sb = xbt_tiles[sc]
                else:
                    xbt_sb = xin.tile([128, 8, S_CHUNK], BF16, tag="xbt")
                    nc.sync.dma_start(
                        out=xbt_sb[:],
                        in_=xbt_v[:, :, sc * S_CHUNK:(sc + 1) * S_CHUNK],
                    )

                yqk_tiles = []
                for st in range(ST_PER_CHUNK):
                    s_lo = st * 128
                    t_glob = sc * ST_PER_CHUNK + st
                    yqk_sb = ywork.tile([128, 1024], F32, tag="yqk")
                    yqk_tiles.append(yqk_sb)
                    for nh in range(3):
                        ps_y = psacc.tile([128, 512], F32, tag="acc")
                        for kb in range(8):
                            nc.tensor.matmul(
                                ps_y[:],
                                lhsT=xbt_sb[:, kb, s_lo:s_lo + 128],
                                rhs=wqkv_sb[:, kb, nh * 512:(nh + 1) * 512],
                                start=(kb == 0),
                                stop=(kb == 7),
                            )
                        if nh < 2:
                            nc.vector.tensor_tensor(
                                out=yqk_sb[:, nh * 512:(nh + 1) * 512],
                                in0=ps_y[:],
                                in1=b_bc[:, nh * 512:(nh + 1) * 512],
                                op=mybir.AluOpType.add,
                            )
                        else:
                            nc.vector.tensor_tensor(
                                out=v_sb[:, t_glob, :],
                                in0=ps_y[:],
                                in1=b_bc[:, 1024:1536],
                                op=mybir.AluOpType.add,
                            )

                # wT accumulation: lhsT=K_h slice, rhs=Q_h slice.  Traced
                # after the full chunk's QKV so the DVE evictions have time
                # to drain before PE needs them.
                for st in range(ST_PER_CHUNK):
                    yqk_sb = yqk_tiles[st]
                    for hl in range(NH):
                        p, g = hl // 2, hl % 2
                        nc.tensor.matmul(
                            psum_wt[g * 64:(g + 1) * 64,
                                    p * 128 + g * 64:p * 128 + (g + 1) * 64],
                            lhsT=yqk_sb[:, 512 + hl * 64:512 + (hl + 1) * 64],
                            rhs=yqk_sb[:, hl * 64:(hl + 1) * 64],
                            # start=True clears has_written for the WHOLE bank
                            # row of the written partitions -> only the first
                            # matmul per partition-half may set it.
                            start=(sc == 0 and st == 0 and hl < 2),
                            stop=(sc == N_SCHUNKS - 1 and st == ST_PER_CHUNK - 1),
                            skip_group_check=True,
                        )

            # ---------------- phase-1.5 weights + G pass ------------------
            w2_sb = const.tile([128, 8, 1024], F32R)
            for jb in range(8):
                nc.scalar.dma_start(out=w2_sb[:, jb, :],
                                    in_=w2t[jb * 128:(jb + 1) * 128, :])
            b2_bc = const.tile([128, 1024], F32)
            nc.gpsimd.dma_start(out=b2_bc[:], in_=b2[:].to_broadcast((128, 1024)))

            # G_{p,half}[e2, n] = sum_j V[half*1024+j, p*128+e2] W2T[j, n]
            # (softmax-independent -> dense PE work before the exp barrier)
            g_sb = attn.tile([128, 16, 512], BF16)      # [(e2), p*4+half*2+nh, n]
            for p in range(4):
                for half in range(2):
                    for nh in range(2):
                        ps_g = psg.tile([128, 512], F32, tag="psg")
                        for jb in range(8):
                            nc.tensor.matmul(
                                ps_g[:],
                                lhsT=v_sb[:, half * 8 + jb,
                                          p * 128:(p + 1) * 128],
                                rhs=w2_sb[:, jb, nh * 512:(nh + 1) * 512],
                                start=(jb == 0),
                                stop=(jb == 7),
                            )
                        nc.vector.tensor_copy(
                            g_sb[:, p * 4 + half * 2 + nh, :], ps_g[:])

            # ---------------- phase 2: exp, Z (overlaps the G pass) -------
            expw_sb = attn.tile([128, 4, 128], BF16)
            nc.vector.memset(expw_sb[:], 0.0)
            for hl in range(NH):
                p, g = hl // 2, hl % 2
                nc.scalar.activation(
                    out=expw_sb[g * 64:(g + 1) * 64, p, g * 64:(g + 1) * 64],
                    in_=psum_wt[g * 64:(g + 1) * 64,
                                p * 128 + g * 64:p * 128 + (g + 1) * 64],
                    func=mybir.ActivationFunctionType.Exp,
                    scale=SCALE,
                )
            ps_z = psf.tile([128, 4], F32, tag="psf")
            rz_sb = attn.tile([128, 4], F32)
            for p in range(4):
                nc.tensor.matmul(
                    ps_z[:, p:p + 1],
                    lhsT=expw_sb[:, p, :],
                    rhs=ones_sb[:],
                    start=(p == 0),
                    stop=(p == 3),
                    skip_group_check=True,
                )
            nc.vector.reciprocal(rz_sb[:], ps_z[:])

            # ---------------- phase 3: F = expw^T x G, store --------------
            for p in range(4):
                for half in range(2):
                    f_sb = fout.tile([128, 1024], F32, tag="f")
                    for nh in range(2):
                        # alternate the two phase-1 pools -> 5-deep rotation
                        idx = (p * 2 + half) * 2 + nh
                        if idx % 5 < 3:
                            ps_f = psacc.tile([128, 512], F32, tag="acc")
                        else:
                            ps_f = psf.tile([128, 512], F32, tag="psf")
                        nc.tensor.matmul(
                            ps_f[:],
                            lhsT=expw_sb[:, p, :],
                            rhs=g_sb[:, p * 4 + half * 2 + nh, :],
                        )
                        # F = psum * rZ (per partition) + b2
                        nc.vector.scalar_tensor_tensor(
                            out=f_sb[:, nh * 512:(nh + 1) * 512],
                            in0=ps_f[:],
                            scalar=rz_sb[:, p:p + 1],
                            in1=b2_bc[:, nh * 512:(nh + 1) * 512],
                            op0=mybir.AluOpType.mult,
                            op1=mybir.AluOpType.add,
                        )
                    # alternate HWDGE queues so the 8 store transfers overlap
                    eng = nc.sync if (p * 2 + half) % 2 == 0 else nc.scalar
                    eng.dma_start(out=out_v[p, :, :, half, :], in_=f_sb[:])

    nc.finalize()
    return nc


_NC_CACHE = None


def _get_nc():
    global _NC_CACHE
    if _NC_CACHE is None:
        _NC_CACHE = build_nc()
    return _NC_CACHE


def _shard_inputs(X, W1, b1, W2, b2):
    X = np.asarray(X, np.float32)
    W1 = np.asarray(W1, np.float32)
    b1 = np.asarray(b1, np.float32)
    W2 = np.asarray(W2, np.float32)
    b2 = np.asarray(b2, np.float32)

    w2t = np.ascontiguousarray(W2.T)
    b2r = np.ascontiguousarray(b2.reshape(1, 1024))
    xbts = [np.ascontiguousarray(X[b].T).astype(ml_dtypes.bfloat16)
            for b in range(B)]

    per_hg = []
    for hg in range(2):
        heads = range(NH * hg, NH * hg + NH)
        rows = np.concatenate(
            [np.arange(h * DH, (h + 1) * DH) for h in heads]
            + [D + np.arange(h * DH, (h + 1) * DH) for h in heads]
            + [2 * D + np.arange(h * DH, (h + 1) * DH) for h in heads])
        wqkvt = np.ascontiguousarray(W1[rows].T).astype(ml_dtypes.bfloat16)
        bqkv = np.ascontiguousarray(b1[rows].reshape(1, 1536))
        per_hg.append((wqkvt, bqkv))

    in_maps = []
    for c in range(8):
        b, hg = c // 2, c % 2
        wqkvt, bqkv = per_hg[hg]
        in_maps.append({
            "xbt": xbts[b], "wqkvt": wqkvt, "bqkv": bqkv,
            "w2t": w2t, "b2": b2r,
        })
    return in_maps


def run(X, W1, b1, W2, b2, **run_kwargs):
    """Returns (full_output, BassKernelResults)."""
    nc = _get_nc()
    in_maps = _shard_inputs(X, W1, b1, W2, b2)
    res = run_bass_kernel_spmd(nc, in_maps, core_ids=list(range(8)), **run_kwargs)
    full = np.empty((B, S, D), np.float32)
    for c in range(8):
        b, hg = c // 2, c % 2
        full[b, hg * 1024:(hg + 1) * 1024, :] = res.results[c]["out"]
    return full, res


def kernel(X, W1, b1, W2, b2):
    return run(X, W1, b1, W2, b2)[0]


# revision 51
# speedup vs baseline: 1.0329x; 1.0329x over previous
"""Trainium2 Bass kernel for nn_Attn_48137993453608.

Module: Y = X@W1.T+b1 -> split Q,K,V -> w = softmax((Q_h^T K_h)/sqrt(S))
        (attention over the DH=64 dim, contracting S) -> out = w @ V_h^T
        -> raw memory-order reshape [B,H,DH,S]->[B,S,D] -> @ W2.T + b2.

Sharding: 8 cores = 4 batch x 2 head-groups (8 heads each). Each core owns a
contiguous [1024, 1024] block of the output (rows i = 128*h + 2*d + (s>=1024)
for its heads), so no collectives are needed.

Key reassociation: the final projection contracts the attention output over
j = s mod 1024, and the attention output is linear in V, so

  F_un[c2, n] = sum_j OT_un[j, c2] W2T[j, n]
              = sum_e expw[e, c2] * G[e, n],
  G_{p,half}[e, n] = sum_j V[half*1024+j, p*128+e] * W2T[j, n].

G is softmax-independent, so nearly all of the output-projection FLOPs run
inside phase 1's dense matmul stream; only a single [128x128]x[128x512]
matmul per (pair, half, nh) remains after the softmax.

Per-core dataflow:
  1. Y[s, :1536] = Xb @ Wqkv.T + b     (Q | K | V columns, local heads)
     Q,K columns feed wT; V columns are stored per s-tile (f32r).
  2. wT_h[e, d] = sum_s K_h[s,e] Q_h[s,d]   (PSUM accum over all s)
  3. G pass (still phase 1 PE work): G = V^T-blocks @ W2T-blocks
  4. expwT = exp(wT / sqrt(S))        (no max-sub: |logits| <= ~6)
     block-diag per head pair; Z via ones-matmul; rZ = 1/Z
  5. F_un = expw2^T-matmul against G; F = F_un * rZ[c2] + b2
  6. scatter F rows to the output block: r = 256*p + 128*g + 2*d + half

Precision: X/Wqkv in bf16 (PE streams 1 row/cycle regardless of dtype, so
bf16 only halves DMA/SBUF); logits are soft (|logit| <= ~6) so the ~0.3%
softmax-weight perturbation is benign.  wT runs in fp32 from the f32 Y,
V/G/W2 path in float32r with one bf16 rounding at G.
"""

import os
import sys

for _p in ("/opt/trn_rl_repo",):
    if _p not in sys.path and os.path.isdir(_p):
        sys.path.insert(0, _p)

import ml_dtypes
import numpy as np

import concourse.bass as bass
import concourse.bacc as bacc
import concourse.mybir as mybir
import concourse.tile as tile
from concourse.bass_utils import run_bass_kernel_spmd

B, S, D, H = 4, 2048, 1024, 16
DH = D // H          # 64
NH = 8               # heads per core
SCALE = 1.0 / float(np.sqrt(np.float32(S)))

F32 = mybir.dt.float32
F32R = mybir.dt.float32r
BF16 = mybir.dt.bfloat16

S_CHUNK = 256                 # s columns of X^T staged per iteration
N_SCHUNKS = S // S_CHUNK      # 8
ST_PER_CHUNK = S_CHUNK // 128 # 2


def build_nc():
    nc = bacc.Bacc("TRN2", target_bir_lowering=False, debug=False)

    xbt = nc.dram_tensor("xbt", [D, S], BF16, kind="ExternalInput")        # X[b].T
    wqkvt = nc.dram_tensor("wqkvt", [D, 1536], BF16, kind="ExternalInput") # [D, Q|K|V rows]
    bqkv = nc.dram_tensor("bqkv", [1, 1536], F32, kind="ExternalInput")
    w2t = nc.dram_tensor("w2t", [D, 1024], F32R, kind="ExternalInput")     # W2.T
    b2 = nc.dram_tensor("b2", [1, 1024], F32, kind="ExternalInput")
    out = nc.dram_tensor("out", [1024, 1024], F32, kind="ExternalOutput")

    xbt_v = xbt[:].rearrange("(kb p) s -> p kb s", p=128)      # [128, 8, 2048]
    # output rows r = 256*p + 128*g + 2*d + half
    out_v = out[:].rearrange("(p g d h) n -> p g d h n", p=4, g=2, d=64, h=2)

    with tile.TileContext(nc) as tc:
        with (
            tc.tile_pool(name="const", bufs=1) as const,
            tc.tile_pool(name="xin", bufs=3) as xin,
            tc.tile_pool(name="ywork", bufs=8) as ywork,
            tc.tile_pool(name="vstore", bufs=1) as vstore,
            tc.tile_pool(name="attn", bufs=1) as attn,
            tc.tile_pool(name="fout", bufs=4) as fout,
            tc.tile_pool(name="psacc", bufs=3, space="PSUM") as psacc,
            tc.tile_pool(name="pswt", bufs=1, space="PSUM") as pswt,
            tc.tile_pool(name="psg", bufs=2, space="PSUM") as psg,
            tc.tile_pool(name="psf", bufs=2, space="PSUM") as psf,
        ):
            # ---------------- phase-1 loads (w2/b2 deferred) --------------
            xbt_tiles = []
            xbt_sb0 = xin.tile([128, 8, S_CHUNK], BF16, tag="xbt")
            nc.sync.dma_start(out=xbt_sb0[:], in_=xbt_v[:, :, 0:S_CHUNK])
            xbt_tiles.append(xbt_sb0)

            # Per-kb loads keep >=3KB contiguous bursts; first QK matmul can
            # start after just the kb=0 block.
            wqkv_sb = const.tile([128, 8, 1536], BF16)
            for kb in range(8):
                nc.scalar.dma_start(out=wqkv_sb[:, kb, :],
                                    in_=wqkvt[kb * 128:(kb + 1) * 128, :])

            b_bc = const.tile([128, 1536], F32)
            nc.gpsimd.dma_start(out=b_bc[:], in_=bqkv[:].to_broadcast((128, 1536)))

            ones_sb = const.tile([128, 1], BF16)
            nc.vector.memset(ones_sb[:], 1.0)

            # V (natural [s, vrow] layout) persists until the G pass;
            # wT accumulates across all s.
            v_sb = vstore.tile([128, 16, 512], F32R)
            psum_wt = pswt.tile([128, 512], F32)        # [e2(g*64+e), pair*128+c2]

            # ---------------- phase 1: QKV projection + wT ----------------
            for sc in range(N_SCHUNKS):
                if sc < len(xbt_tiles):
                    xbt_sb = xbt_tiles[sc]
                else:
                    xbt_sb = xin.tile([128, 8, S_CHUNK], BF16, tag="xbt")
                    nc.sync.dma_start(
                        out=xbt_sb[:],
                        in_=xbt_v[:, :, sc * S_CHUNK:(sc + 1) * S_CHUNK],
                    )

                yqk_tiles = []
                for st in range(ST_PER_CHUNK):
                    s_lo = st * 128
                    t_glob = sc * ST_PER_CHUNK + st
                    yqk_sb = ywork.tile([128, 1024], BF16, tag="yqk")
                    yqk_tiles.append(yqk_sb)
                    for nh in range(3):
                        ps_y = psacc.tile([128, 512], F32, tag="acc")
                        for kb in range(8):
                            nc.tensor.matmul(
                                ps_y[:],
                                lhsT=xbt_sb[:, kb, s_lo:s_lo + 128],
                                rhs=wqkv_sb[:, kb, nh * 512:(nh + 1) * 512],
                                start=(kb == 0),
                                stop=(kb == 7),
                            )
                        if nh < 2:
                            nc.vector.tensor_tensor(
                                out=yqk_sb[:, nh * 512:(nh + 1) * 512],
                                in0=ps_y[:],
                                in1=b_bc[:, nh * 512:(nh + 1) * 512],
                                op=mybir.AluOpType.add,
                            )
                        else:
                            nc.vector.tensor_tensor(
                                out=v_sb[:, t_glob, :],
                                in0=ps_y[:],
                                in1=b_bc[:, 1024:1536],
                                op=mybir.AluOpType.add,
                            )

                # wT accumulation: lhsT=K_h slice, rhs=Q_h slice.  Traced
                # after the full chunk's QKV so the DVE evictions have time
                # to drain before PE needs them.
                for st in range(ST_PER_CHUNK):
                    yqk_sb = yqk_tiles[st]
                    for hl in range(NH):
                        p, g = hl // 2, hl % 2
                        nc.tensor.matmul(
                            psum_wt[g * 64:(g + 1) * 64,
                                    p * 128 + g * 64:p * 128 + (g + 1) * 64],
                            lhsT=yqk_sb[:, 512 + hl * 64:512 + (hl + 1) * 64],
                            rhs=yqk_sb[:, hl * 64:(hl + 1) * 64],
                            # start=True clears has_written for the WHOLE bank
                            # row of the written partitions -> only the first
                            # matmul per partition-half may set it.
                            start=(sc == 0 and st == 0 and hl < 2),
                            stop=(sc == N_SCHUNKS - 1 and st == ST_PER_CHUNK - 1),
                            skip_group_check=True,
                        )

            # ---------------- phase-1.5 weights + G pass ------------------
            w2_sb = const.tile([128, 8, 1024], F32R)
            for jb in range(8):
                nc.scalar.dma_start(out=w2_sb[:, jb, :],
                                    in_=w2t[jb * 128:(jb + 1) * 128, :])
            b2_bc = const.tile([128, 1024], F32)
            nc.gpsimd.dma_start(out=b2_bc[:], in_=b2[:].to_broadcast((128, 1024)))

            # G_{p,half}[e2, n] = sum_j V[half*1024+j, p*128+e2] W2T[j, n]
            # (softmax-independent -> dense PE work before the exp barrier)
            g_sb = attn.tile([128, 16, 512], BF16)      # [(e2), p*4+half*2+nh, n]
            for p in range(4):
                for half in range(2):
                    for nh in range(2):
                        ps_g = psg.tile([128, 512], F32, tag="psg")
                        for jb in range(8):
                            nc.tensor.matmul(
                                ps_g[:],
                                lhsT=v_sb[:, half * 8 + jb,
                                          p * 128:(p + 1) * 128],
                                rhs=w2_sb[:, jb, nh * 512:(nh + 1) * 512],
                                start=(jb == 0),
                                stop=(jb == 7),
                            )
                        nc.vector.tensor_copy(
                            g_sb[:, p * 4 + half * 2 + nh, :], ps_g[:])

            # ---------------- phase 2: exp, Z (overlaps the G pass) -------
            expw_sb = attn.tile([128, 4, 128], BF16)
            nc.vector.memset(expw_sb[:], 0.0)
            for hl in range(NH):
                p, g = hl // 2, hl % 2
                nc.scalar.activation(
                    out=expw_sb[g * 64:(g + 1) * 64, p, g * 64:(g + 1) * 64],
                    in_=psum_wt[g * 64:(g + 1) * 64,
                                p * 128 + g * 64:p * 128 + (g + 1) * 64],
                    func=mybir.ActivationFunctionType.Exp,
                    scale=SCALE,
                )
            ps_z = psf.tile([128, 4], F32, tag="psf")
            rz_sb = attn.tile([128, 4], F32)
            for p in range(4):
                nc.tensor.matmul(
                    ps_z[:, p:p + 1],
                    lhsT=expw_sb[:, p, :],
                    rhs=ones_sb[:],
                    start=(p == 0),
                    stop=(p == 3),
                    skip_group_check=True,
                )
            nc.vector.reciprocal(rz_sb[:], ps_z[:])

            # ---------------- phase 3: F = expw^T x G, store --------------
            for p in range(4):
                for half in range(2):
                    f_sb = fout.tile([128, 1024], F32, tag="f")
                    for nh in range(2):
                        # alternate the two phase-1 pools -> 5-deep rotation
                        idx = (p * 2 + half) * 2 + nh
                        if idx % 5 < 3:
                            ps_f = psacc.tile([128, 512], F32, tag="acc")
                        else:
                            ps_f = psf.tile([128, 512], F32, tag="psf")
                        nc.tensor.matmul(
                            ps_f[:],
                            lhsT=expw_sb[:, p, :],
                            rhs=g_sb[:, p * 4 + half * 2 + nh, :],
                        )
                        # F = psum * rZ (per partition) + b2
                        nc.vector.scalar_tensor_tensor(
                            out=f_sb[:, nh * 512:(nh + 1) * 512],
                            in0=ps_f[:],
                            scalar=rz_sb[:, p:p + 1],
                            in1=b2_bc[:, nh * 512:(nh + 1) * 512],
                            op0=mybir.AluOpType.mult,
                            op1=mybir.AluOpType.add,
                        )
                    # alternate HWDGE queues so the 8 store transfers overlap
                    eng = nc.sync if (p * 2 + half) % 2 == 0 else nc.scalar
                    eng.dma_start(out=out_v[p, :, :, half, :], in_=f_sb[:])

    nc.finalize()
    return nc


_NC_CACHE = None


def _get_nc():
    global _NC_CACHE
    if _NC_CACHE is None:
        _NC_CACHE = build_nc()
    return _NC_CACHE


def _shard_inputs(X, W1, b1, W2, b2):
    X = np.asarray(X, np.float32)
    W1 = np.asarray(W1, np.float32)
    b1 = np.asarray(b1, np.float32)
    W2 = np.asarray(W2, np.float32)
    b2 = np.asarray(b2, np.float32)

    w2t = np.ascontiguousarray(W2.T)
    b2r = np.ascontiguousarray(b2.reshape(1, 1024))
    xbts = [np.ascontiguousarray(X[b].T).astype(ml_dtypes.bfloat16)
            for b in range(B)]

    per_hg = []
    for hg in range(2):
        heads = range(NH * hg, NH * hg + NH)
        rows = np.concatenate(
            [np.arange(h * DH, (h + 1) * DH) for h in heads]
            + [D + np.arange(h * DH, (h + 1) * DH) for h in heads]
            + [2 * D + np.arange(h * DH, (h + 1) * DH) for h in heads])
        wqkvt = np.ascontiguousarray(W1[rows].T).astype(ml_dtypes.bfloat16)
        bqkv = np.ascontiguousarray(b1[rows].reshape(1, 1536))
        per_hg.append((wqkvt, bqkv))

    in_maps = []
    for c in range(8):
        b, hg = c // 2, c % 2
        wqkvt, bqkv = per_hg[hg]
        in_maps.append({
            "xbt": xbts[b], "wqkvt": wqkvt, "bqkv": bqkv,
            "w2t": w2t, "b2": b2r,
        })
    return in_maps


def run(X, W1, b1, W2, b2, **run_kwargs):
    """Returns (full_output, BassKernelResults)."""
    nc = _get_nc()
    in_maps = _shard_inputs(X, W1, b1, W2, b2)
    res = run_bass_kernel_spmd(nc, in_maps, core_ids=list(range(8)), **run_kwargs)
    full = np.empty((B, S, D), np.float32)
    for c in range(8):
        b, hg = c // 2, c % 2
        full[b, hg * 1024:(hg + 1) * 1024, :] = res.results[c]["out"]
    return full, res


def kernel(X, W1, b1, W2, b2):
    return run(X, W1, b1, W2, b2)[0]


# revision 52
# speedup vs baseline: 1.0549x; 1.0213x over previous
"""Trainium2 Bass kernel for nn_Attn_48137993453608.

Module: Y = X@W1.T+b1 -> split Q,K,V -> w = softmax((Q_h^T K_h)/sqrt(S))
        (attention over the DH=64 dim, contracting S) -> out = w @ V_h^T
        -> raw memory-order reshape [B,H,DH,S]->[B,S,D] -> @ W2.T + b2.

Sharding: 8 cores = 4 batch x 2 head-groups (8 heads each). Each core owns a
contiguous [1024, 1024] block of the output (rows i = 128*h + 2*d + (s>=1024)
for its heads), so no collectives are needed.

Key reassociation: the final projection contracts the attention output over
j = s mod 1024, and the attention output is linear in V, so

  F_un[c2, n] = sum_j OT_un[j, c2] W2T[j, n]
              = sum_e expw[e, c2] * G[e, n],
  G_{p,half}[e, n] = sum_j V[half*1024+j, p*128+e] * W2T[j, n].

G is softmax-independent, so nearly all of the output-projection FLOPs run
inside phase 1's dense matmul stream; only a single [128x128]x[128x512]
matmul per (pair, half, nh) remains after the softmax.

Per-core dataflow:
  1. Y[s, :1536] = Xb @ Wqkv.T + b     (Q | K | V columns, local heads)
     Q,K columns feed wT; V columns are stored per s-tile (f32r).
  2. wT_h[e, d] = sum_s K_h[s,e] Q_h[s,d]   (PSUM accum over all s)
  3. G pass (still phase 1 PE work): G = V^T-blocks @ W2T-blocks
  4. expwT = exp(wT / sqrt(S))        (no max-sub: |logits| <= ~6)
     block-diag per head pair; Z via ones-matmul; rZ = 1/Z
  5. F_un = expw2^T-matmul against G; F = F_un * rZ[c2] + b2
  6. scatter F rows to the output block: r = 256*p + 128*g + 2*d + half

Precision: X/Wqkv in bf16 (PE streams 1 row/cycle regardless of dtype, so
bf16 only halves DMA/SBUF); logits are soft (|logit| <= ~6) so the ~0.3%
softmax-weight perturbation is benign.  wT runs in fp32 from the f32 Y,
V/G/W2 path in float32r with one bf16 rounding at G.
"""

import os
import sys

for _p in ("/opt/trn_rl_repo",):
    if _p not in sys.path and os.path.isdir(_p):
        sys.path.insert(0, _p)

import ml_dtypes
import numpy as np

import concourse.bass as bass
import concourse.bacc as bacc
import concourse.mybir as mybir
import concourse.tile as tile
from concourse.bass_utils import run_bass_kernel_spmd

B, S, D, H = 4, 2048, 1024, 16
DH = D // H          # 64
NH = 8               # heads per core
SCALE = 1.0 / float(np.sqrt(np.float32(S)))

F32 = mybir.dt.float32
F32R = mybir.dt.float32r
BF16 = mybir.dt.bfloat16

S_CHUNK = 256                 # s columns of X^T staged per iteration
N_SCHUNKS = S // S_CHUNK      # 8
ST_PER_CHUNK = S_CHUNK // 128 # 2


def build_nc():
    nc = bacc.Bacc("TRN2", target_bir_lowering=False, debug=False)

    xbt = nc.dram_tensor("xbt", [D, S], BF16, kind="ExternalInput")        # X[b].T
    wqkvt = nc.dram_tensor("wqkvt", [D, 1536], BF16, kind="ExternalInput") # [D, Q|K|V rows]
    bqkv = nc.dram_tensor("bqkv", [1, 1536], F32, kind="ExternalInput")
    w2t = nc.dram_tensor("w2t", [D, 1024], BF16, kind="ExternalInput")     # W2.T
    b2 = nc.dram_tensor("b2", [1, 1024], F32, kind="ExternalInput")
    out = nc.dram_tensor("out", [1024, 1024], F32, kind="ExternalOutput")

    xbt_v = xbt[:].rearrange("(kb p) s -> p kb s", p=128)      # [128, 8, 2048]
    # output rows r = 256*p + 128*g + 2*d + half
    out_v = out[:].rearrange("(p g d h) n -> p g d h n", p=4, g=2, d=64, h=2)

    with tile.TileContext(nc) as tc:
        with (
            tc.tile_pool(name="const", bufs=1) as const,
            tc.tile_pool(name="xin", bufs=3) as xin,
            tc.tile_pool(name="ywork", bufs=8) as ywork,
            tc.tile_pool(name="vstore", bufs=1) as vstore,
            tc.tile_pool(name="attn", bufs=1) as attn,
            tc.tile_pool(name="fout", bufs=4) as fout,
            tc.tile_pool(name="psacc", bufs=3, space="PSUM") as psacc,
            tc.tile_pool(name="pswt", bufs=1, space="PSUM") as pswt,
            tc.tile_pool(name="psg", bufs=2, space="PSUM") as psg,
            tc.tile_pool(name="psf", bufs=2, space="PSUM") as psf,
        ):
            # ---------------- phase-1 loads (w2/b2 deferred) --------------
            xbt_tiles = []
            xbt_sb0 = xin.tile([128, 8, S_CHUNK], BF16, tag="xbt")
            nc.sync.dma_start(out=xbt_sb0[:], in_=xbt_v[:, :, 0:S_CHUNK])
            xbt_tiles.append(xbt_sb0)

            # Per-kb loads keep >=3KB contiguous bursts; first QK matmul can
            # start after just the kb=0 block.
            wqkv_sb = const.tile([128, 8, 1536], BF16)
            for kb in range(8):
                nc.scalar.dma_start(out=wqkv_sb[:, kb, :],
                                    in_=wqkvt[kb * 128:(kb + 1) * 128, :])

            b_bc = const.tile([128, 1536], F32)
            nc.gpsimd.dma_start(out=b_bc[:], in_=bqkv[:].to_broadcast((128, 1536)))

            ones_sb = const.tile([128, 1], BF16)
            nc.vector.memset(ones_sb[:], 1.0)

            # V (natural [s, vrow] layout) persists until the G pass;
            # wT accumulates across all s.
            v_sb = vstore.tile([128, 16, 512], BF16)
            psum_wt = pswt.tile([128, 512], F32)        # [e2(g*64+e), pair*128+c2]

            # ---------------- phase 1: QKV projection + wT ----------------
            for sc in range(N_SCHUNKS):
                if sc < len(xbt_tiles):
                    xbt_sb = xbt_tiles[sc]
                else:
                    xbt_sb = xin.tile([128, 8, S_CHUNK], BF16, tag="xbt")
                    nc.sync.dma_start(
                        out=xbt_sb[:],
                        in_=xbt_v[:, :, sc * S_CHUNK:(sc + 1) * S_CHUNK],
                    )

                yqk_tiles = []
                for st in range(ST_PER_CHUNK):
                    s_lo = st * 128
                    t_glob = sc * ST_PER_CHUNK + st
                    yqk_sb = ywork.tile([128, 1024], BF16, tag="yqk")
                    yqk_tiles.append(yqk_sb)
                    for nh in range(3):
                        ps_y = psacc.tile([128, 512], F32, tag="acc")
                        for kb in range(8):
                            nc.tensor.matmul(
                                ps_y[:],
                                lhsT=xbt_sb[:, kb, s_lo:s_lo + 128],
                                rhs=wqkv_sb[:, kb, nh * 512:(nh + 1) * 512],
                                start=(kb == 0),
                                stop=(kb == 7),
                            )
                        if nh < 2:
                            nc.vector.tensor_tensor(
                                out=yqk_sb[:, nh * 512:(nh + 1) * 512],
                                in0=ps_y[:],
                                in1=b_bc[:, nh * 512:(nh + 1) * 512],
                                op=mybir.AluOpType.add,
                            )
                        else:
                            nc.vector.tensor_tensor(
                                out=v_sb[:, t_glob, :],
                                in0=ps_y[:],
                                in1=b_bc[:, 1024:1536],
                                op=mybir.AluOpType.add,
                            )

                # wT accumulation: lhsT=K_h slice, rhs=Q_h slice.  Traced
                # after the full chunk's QKV so the DVE evictions have time
                # to drain before PE needs them.
                for st in range(ST_PER_CHUNK):
                    yqk_sb = yqk_tiles[st]
                    for hl in range(NH):
                        p, g = hl // 2, hl % 2
                        nc.tensor.matmul(
                            psum_wt[g * 64:(g + 1) * 64,
                                    p * 128 + g * 64:p * 128 + (g + 1) * 64],
                            lhsT=yqk_sb[:, 512 + hl * 64:512 + (hl + 1) * 64],
                            rhs=yqk_sb[:, hl * 64:(hl + 1) * 64],
                            # start=True clears has_written for the WHOLE bank
                            # row of the written partitions -> only the first
                            # matmul per partition-half may set it.
                            start=(sc == 0 and st == 0 and hl < 2),
                            stop=(sc == N_SCHUNKS - 1 and st == ST_PER_CHUNK - 1),
                            skip_group_check=True,
                        )

            # ---------------- phase-1.5 weights + G pass ------------------
            w2_sb = const.tile([128, 8, 1024], BF16)
            for jb in range(8):
                nc.scalar.dma_start(out=w2_sb[:, jb, :],
                                    in_=w2t[jb * 128:(jb + 1) * 128, :])
            b2_bc = const.tile([128, 1024], F32)
            nc.gpsimd.dma_start(out=b2_bc[:], in_=b2[:].to_broadcast((128, 1024)))

            # G_{p,half}[e2, n] = sum_j V[half*1024+j, p*128+e2] W2T[j, n]
            # (softmax-independent -> dense PE work before the exp barrier)
            g_sb = attn.tile([128, 16, 512], BF16)      # [(e2), p*4+half*2+nh, n]
            for p in range(4):
                for half in range(2):
                    for nh in range(2):
                        ps_g = psg.tile([128, 512], F32, tag="psg")
                        for jb in range(8):
                            nc.tensor.matmul(
                                ps_g[:],
                                lhsT=v_sb[:, half * 8 + jb,
                                          p * 128:(p + 1) * 128],
                                rhs=w2_sb[:, jb, nh * 512:(nh + 1) * 512],
                                start=(jb == 0),
                                stop=(jb == 7),
                            )
                        nc.vector.tensor_copy(
                            g_sb[:, p * 4 + half * 2 + nh, :], ps_g[:])

            # ---------------- phase 2: exp, Z (overlaps the G pass) -------
            expw_sb = attn.tile([128, 4, 128], BF16)
            nc.vector.memset(expw_sb[:], 0.0)
            for hl in range(NH):
                p, g = hl // 2, hl % 2
                nc.scalar.activation(
                    out=expw_sb[g * 64:(g + 1) * 64, p, g * 64:(g + 1) * 64],
                    in_=psum_wt[g * 64:(g + 1) * 64,
                                p * 128 + g * 64:p * 128 + (g + 1) * 64],
                    func=mybir.ActivationFunctionType.Exp,
                    scale=SCALE,
                )
            ps_z = psf.tile([128, 4], F32, tag="psf")
            rz_sb = attn.tile([128, 4], F32)
            for p in range(4):
                nc.tensor.matmul(
                    ps_z[:, p:p + 1],
                    lhsT=expw_sb[:, p, :],
                    rhs=ones_sb[:],
                    start=(p == 0),
                    stop=(p == 3),
                    skip_group_check=True,
                )
            nc.vector.reciprocal(rz_sb[:], ps_z[:])

            # ---------------- phase 3: F = expw^T x G, store --------------
            for p in range(4):
                for half in range(2):
                    f_sb = fout.tile([128, 1024], F32, tag="f")
                    for nh in range(2):
                        # alternate the two phase-1 pools -> 5-deep rotation
                        idx = (p * 2 + half) * 2 + nh
                        if idx % 5 < 3:
                            ps_f = psacc.tile([128, 512], F32, tag="acc")
                        else:
                            ps_f = psf.tile([128, 512], F32, tag="psf")
                        nc.tensor.matmul(
                            ps_f[:],
                            lhsT=expw_sb[:, p, :],
                            rhs=g_sb[:, p * 4 + half * 2 + nh, :],
                        )
                        # F = psum * rZ (per partition) + b2
                        nc.vector.scalar_tensor_tensor(
                            out=f_sb[:, nh * 512:(nh + 1) * 512],
                            in0=ps_f[:],
                            scalar=rz_sb[:, p:p + 1],
                            in1=b2_bc[:, nh * 512:(nh + 1) * 512],
                            op0=mybir.AluOpType.mult,
                            op1=mybir.AluOpType.add,
                        )
                    # alternate HWDGE queues so the 8 store transfers overlap
                    eng = nc.sync if (p * 2 + half) % 2 == 0 else nc.scalar
                    eng.dma_start(out=out_v[p, :, :, half, :], in_=f_sb[:])

    nc.finalize()
    return nc


_NC_CACHE = None


def _get_nc():
    global _NC_CACHE
    if _NC_CACHE is None:
        _NC_CACHE = build_nc()
    return _NC_CACHE


def _shard_inputs(X, W1, b1, W2, b2):
    X = np.asarray(X, np.float32)
    W1 = np.asarray(W1, np.float32)
    b1 = np.asarray(b1, np.float32)
    W2 = np.asarray(W2, np.float32)
    b2 = np.asarray(b2, np.float32)

    w2t = np.ascontiguousarray(W2.T).astype(ml_dtypes.bfloat16)
    b2r = np.ascontiguousarray(b2.reshape(1, 1024))
    xbts = [np.ascontiguousarray(X[b].T).astype(ml_dtypes.bfloat16)
            for b in range(B)]

    per_hg = []
    for hg in range(2):
        heads = range(NH * hg, NH * hg + NH)
        rows = np.concatenate(
            [np.arange(h * DH, (h + 1) * DH) for h in heads]
            + [D + np.arange(h * DH, (h + 1) * DH) for h in heads]
            + [2 * D + np.arange(h * DH, (h + 1) * DH) for h in heads])
        wqkvt = np.ascontiguousarray(W1[rows].T).astype(ml_dtypes.bfloat16)
        bqkv = np.ascontiguousarray(b1[rows].reshape(1, 1536))
        per_hg.append((wqkvt, bqkv))

    in_maps = []
    for c in range(8):
        b, hg = c // 2, c % 2
        wqkvt, bqkv = per_hg[hg]
        in_maps.append({
            "xbt": xbts[b], "wqkvt": wqkvt, "bqkv": bqkv,
            "w2t": w2t, "b2": b2r,
        })
    return in_maps


def run(X, W1, b1, W2, b2, **run_kwargs):
    """Returns (full_output, BassKernelResults)."""
    nc = _get_nc()
    in_maps = _shard_inputs(X, W1, b1, W2, b2)
    res = run_bass_kernel_spmd(nc, in_maps, core_ids=list(range(8)), **run_kwargs)
    full = np.empty((B, S, D), np.float32)
    for c in range(8):
        b, hg = c // 2, c % 2
        full[b, hg * 1024:(hg + 1) * 1024, :] = res.results[c]["out"]
    return full, res


def kernel(X, W1, b1, W2, b2):
    return run(X, W1, b1, W2, b2)[0]
